# revision 1
# baseline (speedup 1.0000x reference)
"""CPD block (1x1 conv -> depthwise 1x3 -> depthwise 3x1 + bias) on 8 trn2 cores.

Contract: kernel(**inputs) takes FULL inputs (x:[8,64,256,256] f32, w1:[64,64],
wh:[64,3], wv:[64,3], bias:[64]) and returns the FULL output [8,64,256,256] f32.

Strategy
--------
Data-parallel over batch: 1 image per core, 8 cores, no collectives.

The input is zero-padded on the host to [64, 258, 258] so the kernel has no
boundary special cases.  The 1x1 conv and the horizontal 1x3 depthwise conv
are fused into 3 "tap" matmuls over the in-channel dim
(W_dx[o,c] = w1[o,c]*wh[o,dx]) accumulated in PSUM; the taps read
column-shifted views of the padded x tile.  The vertical 3x1 conv + bias
runs on ACT (center tap, Identity(z*wv1+bias) with per-partition
scale/bias) and DVE (two scalar_tensor_tensor fused mul-adds for the
up/down taps).

Layout: the 256-row image is split into two 128-row halves stacked on the
128 SBUF partitions (partition = half*64 + channel), so DMA and the vector
engines run at full 128-partition width.  Each tap matmul uses K=128
block-diagonal weights (diag(W_dx, W_dx)) so one N=512 instruction computes
both halves at once.  Matmuls run in float16 (~3e-4 rel err).
"""

import numpy as np

import concourse.bacc as bacc
import concourse.mybir as mybir
from concourse.tile import TileContext
from concourse.bass_utils import run_bass_kernel_spmd

B, C, O = 8, 64, 64
H, W = 256, 256
WP = W + 2             # padded width
N_CORES = 8
HALF = H // 2          # rows per half-image
SEG = 16               # output rows per half per segment
NSEG = HALF // SEG
HB = 6                 # output rows per PSUM block (z block = HB+2 rows)

F16 = mybir.dt.float16


ZB = 4    # z rows per PSUM block (2 banks); no halo recompute
VB = 8    # output rows per v-conv piece


def _kernel_body(tc, out, x, w, v, reps=1, taps=(0, 1, 2), vconv=True,
                 warmup=0, tap_outer=False):
    nc = tc.nc
    f32 = mybir.dt.float32
    mult, add = mybir.AluOpType.mult, mybir.AluOpType.add
    ZR = SEG + 2  # z rows per segment

    with (
        tc.tile_pool(name="const", bufs=1) as cpool,
        tc.tile_pool(name="xp", bufs=3) as xpool,
        tc.tile_pool(name="op", bufs=2) as opool,
        tc.tile_pool(name="zs", bufs=2) as zspool,
        tc.tile_pool(name="tp", bufs=2) as tpool,
        tc.tile_pool(name="zp", bufs=4, space="PSUM") as zpool,
    ):
        w_sb = cpool.tile([128, 3 * 128], F16)
        nc.sync.dma_start(out=w_sb, in_=w)
        v_sb = cpool.tile([128, 4], f32)
        nc.sync.dma_start(out=v_sb, in_=v)

        if warmup:
            # Dummy matmuls while the first segment DMA is in flight: ramps
            # the PE HAM clock gate to 8/8 before the real work starts.
            wz = zpool.tile([128, ZB * W], f32, tag="zt")
            for i in range(warmup):
                nc.tensor.matmul(
                    out=wz[:, 0:384],
                    lhsT=w_sb[:, 0:128],
                    rhs=w_sb,
                    start=(i == 0),
                    stop=(i == warmup - 1),
                )

        # Partition convention: p = 2*c + hh (channel-major, half fastest).
        # out viewed as [c, hh, hr, w] then flat-paired with the [128, SEG, W]
        # SBUF tile: flat orders match exactly, one full-width DMA per segment.
        orr = out.rearrange("c (hh hr) w -> c hh hr w", hh=2)
        for rep in range(reps):
          for s in range(NSEG):
            r0 = s * SEG  # segment start row, half-local coords
            # x is host-prepped as [128, HALF+2, WP]: partition p = 2c+hh
            # already carries that half's rows (with halo): one full-width DMA.
            xt = xpool.tile([128, ZR, WP], F16, tag="xt")
            nc.sync.dma_start(out=xt, in_=x[:, r0 : r0 + ZR, :])

            ot = opool.tile([128, SEG, W], f32, tag="ot")
            # z for the whole segment, evacuated from PSUM to SBUF by ACT
            zseg = zspool.tile([128, ZR, W], f32, tag="zseg")

            # 1x1 conv + horizontal conv: 3 taps accumulated in PSUM, in
            # ZB-row blocks with NO halo recompute; ACT copies each block out.
            # z row i = half row r0-1+i = x tile row i.
            for b0 in range(0, ZR, ZB):
                zb = min(ZB, ZR - b0)
                zt = zpool.tile([128, ZB * W], f32, tag="zt")
                for j in range(zb // 2):  # 2-row chunks (one PSUM bank)
                    xr = b0 + 2 * j
                    for i, dx in enumerate(taps):
                        nc.tensor.matmul(
                            out=zt[:, j * 512 : (j + 1) * 512],
                            lhsT=w_sb[:, dx * 128 : (dx + 1) * 128],
                            rhs=xt[:, xr : xr + 2, dx : dx + W],
                            start=(i == 0),
                            stop=(i == len(taps) - 1),
                        )
                nc.scalar.copy(
                    out=zseg[:, b0 : b0 + zb, :],
                    in_=zt.rearrange("p (r w) -> p r w", w=W)[:, :zb, :],
                )

            # Vertical conv + bias in VB-row pieces from SBUF z:
            #   u[t] = wv0*z[t] + wv1*z[t+1] + wv2*z[t+2] + bias
            # center tap + bias on ACT (Identity with per-partition
            # scale/bias), up/down taps as fused mul-adds on DVE.
            for p0 in range(0, SEG, VB):
                tt = tpool.tile([128, VB, W], f32, tag="tt")
                t2 = tpool.tile([128, VB, W], f32, tag="t2")
                nc.scalar.activation(
                    out=tt if vconv else ot[:, p0 : p0 + VB, :],
                    in_=zseg[:, p0 + 1 : p0 + 1 + VB, :],
                    func=mybir.ActivationFunctionType.Identity,
                    scale=v_sb[:, 1:2],
                    bias=v_sb[:, 3:4],
                )
                if not vconv:
                    continue
                nc.vector.scalar_tensor_tensor(
                    out=t2,
                    in0=zseg[:, p0 : p0 + VB, :],
                    scalar=v_sb[:, 0:1],
                    in1=tt,
                    op0=mult,
                    op1=add,
                )
                nc.vector.scalar_tensor_tensor(
                    out=ot[:, p0 : p0 + VB, :],
                    in0=zseg[:, p0 + 2 : p0 + 2 + VB, :],
                    scalar=v_sb[:, 2:3],
                    in1=t2,
                    op0=mult,
                    op1=add,
                )

            nc.scalar.dma_start(out=orr[:, :, r0 : r0 + SEG, :], in_=ot)


_CACHE = {}


def _build(reps=1, taps=(0, 1, 2), vconv=True, warmup=0, tap_outer=False):
    key = ("nc", reps, taps, vconv, warmup, tap_outer)
    if key in _CACHE:
        return _CACHE[key]
    nc = bacc.Bacc("TRN2", target_bir_lowering=False, debug=False)
    xd = nc.dram_tensor("x", [128, HALF + 2, WP], F16, kind="ExternalInput").ap()
    wd = nc.dram_tensor("w", [128, 3 * 128], F16, kind="ExternalInput").ap()
    vd = nc.dram_tensor("v", [128, 4], mybir.dt.float32, kind="ExternalInput").ap()
    od = nc.dram_tensor("out", [C, H, W], mybir.dt.float32, kind="ExternalOutput").ap()
    with TileContext(nc) as tc:
        _kernel_body(tc, od, xd, wd, vd, reps=reps, taps=taps, vconv=vconv,
                     warmup=warmup, tap_outer=tap_outer)
    nc.compile()
    _CACHE[key] = nc
    return nc


def prep_inputs(x, w1, wh, wv, bias):
    """Host-side input prep shared by kernel() and benchmarks."""
    x = np.asarray(x, dtype=np.float32)
    w1 = np.asarray(w1, dtype=np.float32)
    wh = np.asarray(wh, dtype=np.float32)
    wv = np.asarray(wv, dtype=np.float32)
    bias = np.asarray(bias, dtype=np.float32)

    # Host-side zero pad, then split into two 128-row halves (with one halo
    # row on each side) stacked on the partition axis: [B, 128, HALF+2, WP].
    xpad = np.zeros((B, C, H + 2, WP), np.float16)
    xpad[:, :, 1 : H + 1, 1 : W + 1] = x.astype(np.float16)
    xp = np.empty((B, C, 2, HALF + 2, WP), np.float16)
    for hh in range(2):
        xp[:, :, hh] = xpad[:, :, hh * HALF : hh * HALF + HALF + 2, :]
    xp = xp.reshape(B, 128, HALF + 2, WP)  # partition p = 2*c + hh

    # Fold the horizontal conv into the 1x1 and build K=128 block-diagonal
    # taps: lhsT_dx = diag(W_dx.T, W_dx.T) with W_dx[o,c] = w1[o,c]*wh[o,dx].
    w_np = np.zeros((128, 3 * 128), np.float16)
    for dx in range(3):
        blk = (w1 * wh[:, dx : dx + 1]).T.astype(np.float16)  # [c, o]
        wb = np.zeros((C, 2, O, 2), np.float16)
        wb[:, 0, :, 0] = blk
        wb[:, 1, :, 1] = blk
        w_np[:, dx * 128 : (dx + 1) * 128] = wb.reshape(128, 128)
    # Per-partition vertical-tap weights + bias: [wv0, wv1, wv2, bias]
    v_np = np.stack([wv[:, 0], wv[:, 1], wv[:, 2], bias], axis=1)
    v_np = np.repeat(v_np, 2, axis=0).astype(np.float32)  # p = 2*c + hh
    return xp, w_np, v_np


def kernel(x, w1, wh, wv, bias, _results_out=None):
    xp, w_np, v_np = prep_inputs(x, w1, wh, wv, bias)
    nc = _build()
    in_maps = [{"x": xp[b], "w": w_np, "v": v_np} for b in range(B)]
    res = run_bass_kernel_spmd(nc, in_maps, list(range(N_CORES)))
    if _results_out is not None:
        _results_out.append(res)
    return np.stack([res.results[b]["out"] for b in range(B)], axis=0)

